# revision 1
# baseline (speedup 1.0000x reference)
"""ComplexityAwareAttention Trainium2 Bass kernel.

Sharding: 8 cores = 2 batches x 4 head-groups (3 heads each). Each core
computes q/k/v projections for its 3 heads, masked-key-gathered attention
(keys with attention_mask==0 are removed on host - softmax over the kept
keys is mathematically identical), and a partial output projection
(2048, 768). Host sums the 4 partials per batch and adds the fused output
bias (bo + Wo @ bv).

Device layouts (all chosen so no on-chip transposes are needed):
  x.T      [768, 2048]  d on partitions       (q projection rhs)
  xk.T     [768, n_k]   gathered+padded keys  (k/v projection inputs)
  q.T,k.T  [64h, seq]   head-dim on partitions
  v_aug    [keys, 65]   natural; col 64 = 1.0 for real keys, 0 for pads
                        -> softmax denominator falls out of the o.T matmul
  s.T      [keys, q]    scores transposed; exp on ScalarE out of PSUM
  o.T      [65, q]      un-normalized attended + denominator row
Normalization: recip(denom) -> K=1 ones matmul broadcast [1,q]->[64,q] in
PSUM -> single DVE multiply while moving o.T to SBUF. No max-subtraction
in softmax (|scores| << 80, exp cannot overflow in f32).
"""

import math
import os
from contextlib import ExitStack

import numpy as np

import concourse.bass as bass
from concourse import bacc
import concourse.mybir as mybir
import concourse.tile as tile
from concourse.bass import ds, ts
from concourse.bass_utils import run_bass_kernel_spmd

F32 = mybir.dt.float32
F32R = mybir.dt.float32r
AFT = mybir.ActivationFunctionType

B = 2
S = 2048
D = 768
H = 12
HD = 64
NH = 3  # heads per core
KT_D = D // 128  # 6 contraction tiles over d_model

LAST_EXEC_TIME_NS = None
LAST_RESULTS = None


def _free_chunks(n):
    """Split n into bank-friendly matmul free-dim chunks (<=512, >=256 when
    possible, each chunk allocated at a PSUM-tile start so it never crosses a
    bank boundary)."""
    out = []
    rem = n
    while rem > 0:
        if rem >= 512:
            take = 384 if rem % 512 == 128 else 512
        else:
            take = rem
        out.append(take)
        rem -= take
    return out


def build_nc(n_k):
    nk_t = n_k // 128
    nc = bacc.Bacc(None, target_bir_lowering=False)

    d_xT = nc.dram_tensor("xT", (D, S), F32R, kind="ExternalInput")
    d_xkT = nc.dram_tensor("xkT", (D, n_k), F32R, kind="ExternalInput")
    d_wqT = nc.dram_tensor("wqT", (D, 192), F32R, kind="ExternalInput")
    d_wkT = nc.dram_tensor("wkT", (D, 192), F32R, kind="ExternalInput")
    d_wvT = nc.dram_tensor("wvT", (D, 256), F32R, kind="ExternalInput")
    d_woR = nc.dram_tensor("woR", (256, D), F32R, kind="ExternalInput")
    d_bq = nc.dram_tensor("bq", (128, 2), F32, kind="ExternalInput")
    d_bk = nc.dram_tensor("bk", (128, 2), F32, kind="ExternalInput")
    d_vcol = nc.dram_tensor("vcol", (128, nk_t), F32R, kind="ExternalInput")
    d_out = nc.dram_tensor("out", (S, D), F32, kind="ExternalOutput")

    with ExitStack() as ctx:
        tc = ctx.enter_context(tile.TileContext(nc))
        singles = ctx.enter_context(tc.tile_pool(name="singles", bufs=1))
        expp = ctx.enter_context(tc.tile_pool(name="expp", bufs=3))
        outp = ctx.enter_context(tc.tile_pool(name="outp", bufs=2))
        rowp = ctx.enter_context(tc.tile_pool(name="rowp", bufs=2))
        psp = ctx.enter_context(tc.tile_pool(name="psp", bufs=2, space="PSUM"))
        oaccp = ctx.enter_context(tc.tile_pool(name="oaccp", bufs=2, space="PSUM"))

        # ---- load weights / small tensors ----
        sb_wqT = singles.tile([128, KT_D, 192], F32R)
        sb_wkT = singles.tile([128, KT_D, 192], F32R)
        sb_wvT = singles.tile([128, KT_D, 256], F32R)
        nc.sync.dma_start(out=sb_wqT, in_=d_wqT.rearrange("(t p) m -> p t m", p=128))
        nc.scalar.dma_start(out=sb_wkT, in_=d_wkT.rearrange("(t p) m -> p t m", p=128))
        nc.scalar.dma_start(out=sb_wvT, in_=d_wvT.rearrange("(t p) m -> p t m", p=128))
        sb_woR = singles.tile([128, 2, D], F32R)
        nc.gpsimd.dma_start(out=sb_woR, in_=d_woR.rearrange("(t p) e -> p t e", p=128))
        sb_bq = singles.tile([128, 2], F32)
        sb_bk = singles.tile([128, 2], F32)
        nc.gpsimd.dma_start(out=sb_bq, in_=d_bq[:, :])
        nc.gpsimd.dma_start(out=sb_bk, in_=d_bk[:, :])

        # v_aug: [keys(128 x nk_t), head, 65]; col 64 = real-key indicator
        sb_v = singles.tile([128, NH, nk_t, 65], F32R)
        for h in range(NH):
            nc.gpsimd.dma_start(out=sb_v[:, h, :, 64:65], in_=d_vcol[:, :])

        # ---- load activations ----
        sb_xT = singles.tile([128, KT_D, S], F32R)
        sb_xkT = singles.tile([128, KT_D, n_k], F32R)
        for t in range(KT_D):
            nc.sync.dma_start(out=sb_xT[:, t, :], in_=d_xT[ts(t, 128), :])
            (nc.scalar if t < 3 else nc.gpsimd).dma_start(
                out=sb_xkT[:, t, :], in_=d_xkT[ts(t, 128), :]
            )

        # ---- projections ----
        # q.T / k.T packed [192, seq]: slot 0 = heads 0,1 (128 rows),
        # slot 1 = head 2 (64 rows)
        sb_qT = singles.tile([128, 2, S], F32R)
        sb_kT = singles.tile([128, 2, n_k], F32R)
        sb_onT = singles.tile([128, 2, S], F32R)

        kchunks = _free_chunks(n_k)

        for m, rows in enumerate((128, 64)):
            msl = ds(m * 128, rows)
            # q.T
            for qc in range(0, S, 1024):
                ps = psp.tile([128, 1024], F32, tag="ps")
                for half in range(2):
                    sl = ts(half, 512)
                    for kt in range(KT_D):
                        nc.tensor.matmul(
                            ps[:rows, sl],
                            sb_wqT[:, kt, msl],
                            sb_xT[:, kt, ds(qc + half * 512, 512)],
                            start=(kt == 0),
                            stop=(kt == KT_D - 1),
                        )
                nc.vector.tensor_scalar_add(
                    out=sb_qT[:rows, m, ds(qc, 1024)],
                    in0=ps[:rows, :],
                    scalar1=sb_bq[:rows, m : m + 1],
                )
            # k.T
            off = 0
            for ck in kchunks:
                ps = psp.tile([128, ck], F32, tag="ps")
                for kt in range(KT_D):
                    nc.tensor.matmul(
                        ps[:rows, :],
                        sb_wkT[:, kt, msl],
                        sb_xkT[:, kt, ds(off, ck)],
                        start=(kt == 0),
                        stop=(kt == KT_D - 1),
                    )
                nc.vector.tensor_scalar_add(
                    out=sb_kT[:rows, m, ds(off, ck)],
                    in0=ps[:rows, :],
                    scalar1=sb_bk[:rows, m : m + 1],
                )
                off += ck

        # v (natural layout, all 3 heads at once; N padded to 256 for fp32r)
        for kt2 in range(nk_t):
            ps = psp.tile([128, 256], F32, tag="ps")
            for kt in range(KT_D):
                nc.tensor.matmul(
                    ps,
                    sb_xkT[:, kt, ts(kt2, 128)],
                    sb_wvT[:, kt, :],
                    start=(kt == 0),
                    stop=(kt == KT_D - 1),
                )
            nc.vector.tensor_copy(
                out=sb_v[:, :, kt2, 0:64],
                in_=ps[:, 0:192].rearrange("p (h d) -> p h d", h=NH),
            )

        # ---- attention (half-major) + finals interleaved per half ----
        for half in range(2):
            for h in range(NH):
                qrow = (h % 2) * 64
                qslot = h // 2
                oacc = oaccp.tile([65, 1024], F32, tag="oacc")
                for kt2 in range(nk_t):
                    sT = psp.tile([128, 1024], F32, tag="ps")
                    for qc in range(2):
                        nc.tensor.matmul(
                            sT[:, ts(qc, 512)],
                            sb_kT[ds(qrow, 64), qslot, ts(kt2, 128)],
                            sb_qT[ds(qrow, 64), qslot, ds(half * 1024 + qc * 512, 512)],
                            start=True,
                            stop=True,
                        )
                    et = expp.tile([128, 1024], F32R, tag="exp")
                    nc.scalar.activation(et, sT, AFT.Exp)
                    for qc in range(2):
                        nc.tensor.matmul(
                            oacc[:, ts(qc, 512)],
                            sb_v[:, h, kt2, :],
                            et[:, ts(qc, 512)],
                            start=(kt2 == 0),
                            stop=(kt2 == nk_t - 1),
                        )
                # normalize: o.T[0:64] / denom (row 64), into sb_onT
                lrow = rowp.tile([1, 1024], F32, tag="lrow")
                nc.scalar.activation(lrow, oacc[64:65, :], AFT.Ln)
                rrow = rowp.tile([1, 1024], F32, tag="rrow")
                nc.scalar.activation(rrow, lrow, AFT.Exp, scale=-1.0)
                bcast = rowp.tile([64, 1024], F32, tag="bcast")
                nc.gpsimd.partition_broadcast(bcast, rrow)
                nc.vector.tensor_mul(
                    out=sb_onT[ds(qrow, 64), qslot, ds(half * 1024, 1024)],
                    in0=oacc[0:64, :],
                    in1=bcast,
                )
            # output projection for this half: partial[q, e] = sum_h o_h @ WoR_h
            for qt in range(half * 8, half * 8 + 8):
                ps = psp.tile([128, D], F32, tag="ps")
                for eoff, ech in ((0, 512), (512, 256)):
                    nc.tensor.matmul(
                        ps[:, ds(eoff, ech)],
                        sb_onT[:, 0, ts(qt, 128)],
                        sb_woR[:, 0, ds(eoff, ech)],
                        start=True,
                        stop=False,
                    )
                    nc.tensor.matmul(
                        ps[:, ds(eoff, ech)],
                        sb_onT[0:64, 1, ts(qt, 128)],
                        sb_woR[0:64, 1, ds(eoff, ech)],
                        start=False,
                        stop=True,
                    )
                ot = outp.tile([128, D], F32, tag="out")
                nc.vector.tensor_copy(out=ot, in_=ps)
                nc.sync.dma_start(out=d_out[ts(qt, 128), :], in_=ot)

    nc.compile()
    return nc


def kernel(
    hidden_states,
    complexity_scores,
    attention_mask,
    Wq,
    bq,
    Wk,
    bk,
    Wv,
    bv,
    Wo,
    bo,
    emb_table,
    comp_scaling,
):
    global LAST_EXEC_TIME_NS, LAST_RESULTS
    hs = np.asarray(hidden_states, np.float32)
    cs = np.asarray(complexity_scores).astype(np.int64)
    am = np.asarray(attention_mask)
    Wq = np.asarray(Wq, np.float32)
    bq = np.asarray(bq, np.float32)
    Wk = np.asarray(Wk, np.float32)
    bk = np.asarray(bk, np.float32)
    Wv = np.asarray(Wv, np.float32)
    bv = np.asarray(bv, np.float32)
    Wo = np.asarray(Wo, np.float32)
    bo = np.asarray(bo, np.float32)
    emb_table = np.asarray(emb_table, np.float32)
    comp_scaling = np.asarray(comp_scaling, np.float32)

    # per-head score scale (identical across batch: mean over batch of embs)
    embs = emb_table[cs]  # (B, H)
    scal = comp_scaling * embs.mean(axis=0)  # (H,)
    c = (scal / math.sqrt(HD)).astype(np.float32)

    # gather unmasked keys per batch; pad to a common multiple of 128
    idx = [np.nonzero(am[b] != 0)[0] for b in range(B)]
    n_max = max(1, max(len(i) for i in idx))
    n_k = max(256, ((n_max + 127) // 128) * 128)
    nk_t = n_k // 128

    xT = [np.ascontiguousarray(hs[b].T) for b in range(B)]
    xkT = []
    vcol = []
    for b in range(B):
        t = np.zeros((D, n_k), np.float32)
        t[:, : len(idx[b])] = hs[b][idx[b]].T
        xkT.append(t)
        v = np.zeros((n_k,), np.float32)
        v[: len(idx[b])] = 1.0
        vcol.append(np.ascontiguousarray(v.reshape(nk_t, 128).T))

    WqT = Wq.T  # (d_in, e_out)
    WkT = Wk.T
    WvT = Wv.T
    WoT = np.ascontiguousarray(Wo.T)  # rows = attended feature d

    def pack_bias(vec):  # (192,) -> (128, 2)
        out = np.zeros((128, 2), np.float32)
        out[:, 0] = vec[:128]
        out[:64, 1] = vec[128:]
        return out

    in_maps = []
    for core in range(8):
        b = core // 4
        heads = [3 * (core % 4) + j for j in range(NH)]
        cols = np.concatenate([np.arange(h * HD, (h + 1) * HD) for h in heads])
        cscale = np.repeat(c[heads], HD)  # (192,)
        wqT_c = np.ascontiguousarray(WqT[:, cols] * cscale[None, :])
        bq_c = bq[cols] * cscale
        wkT_c = np.ascontiguousarray(WkT[:, cols])
        bk_c = bk[cols]
        wvT_c = np.zeros((D, 256), np.float32)
        wvT_c[:, :192] = WvT[:, cols]
        woR_c = np.zeros((256, D), np.float32)
        woR_c[:192] = WoT[cols, :]
        in_maps.append(
            {
                "xT": xT[b],
                "xkT": xkT[b],
                "wqT": wqT_c,
                "wkT": wkT_c,
                "wvT": wvT_c,
                "woR": np.ascontiguousarray(woR_c),
                "bq": pack_bias(bq_c),
                "bk": pack_bias(bk_c),
                "vcol": vcol[b],
            }
        )

    nc = build_nc(n_k)
    trace = os.environ.get("KERNEL_TRACE", "0") == "1"
    res = run_bass_kernel_spmd(nc, in_maps, core_ids=list(range(8)), trace=trace)
    LAST_EXEC_TIME_NS = res.exec_time_ns
    LAST_RESULTS = res

    partials = [r["out"] for r in res.results]
    bo_eff = (bo + Wo @ bv).astype(np.float64)
    out = np.empty((B, S, D), np.float32)
    for b in range(B):
        acc = np.zeros((S, D), np.float64)
        for g in range(4):
            acc += partials[4 * b + g].astype(np.float64)
        out[b] = (acc + bo_eff[None, :]).astype(np.float32)
    return out



# revision 4
# speedup vs baseline: 1.5395x; 1.5395x over previous
"""ComplexityAwareAttention Trainium2 Bass kernel.

Sharding: 8 cores = 2 batches x 4 head-groups (3 heads each). Each core
computes q/k/v projections for its 3 heads, masked-key-gathered attention
(keys with attention_mask==0 are removed on host - softmax over the kept
keys is mathematically identical), and a partial output projection
(2048, 768). Host sums the 4 partials per batch and adds the fused output
bias (bo + Wo @ bv).

Numerics: fp16 for x / projection weights / q / k / onT / Wo / output
partials (validated 1.5e-3 rel err vs the f32 reference); f32r for the
exp'd scores and v (exp output can exceed fp16 range); f32 PSUM.
fp16 halves all DMA traffic vs f32 and streams at 1 cycle/row on the PE
at any ap size.

Device layouts (all chosen so no on-chip transposes are needed):
  x.T      [128, 2, 6, 1024] d on partitions, col-chunked (q proj rhs)
  xk.T     [128, nkc, 6, 512] gathered+padded keys (k/v proj inputs)
  q.T,k.T  [64h, seq]   head-dim on partitions
  v_aug    [keys, 65]   natural; col 64 = 1.0 for real keys, 0 for pads
                        -> softmax denominator falls out of the o.T matmul
  s.T      [keys, q]    scores transposed; exp on ScalarE out of PSUM
  o.T      [65, q]      un-normalized attended + denominator row
All HBM tensors are pre-packed on host into the exact SBUF layout
(partition-major) so every DMA moves large contiguous per-partition runs.
Normalization: reciprocal_approx_fast (DVE) -> partition_broadcast (Pool)
-> DVE multiply; only the Exp activation table is ever loaded (preloaded
by a dummy exp at t=0). No max-subtraction in softmax (|scores| << 80,
exp cannot overflow in f32).
"""

import math
import os
from contextlib import ExitStack

import numpy as np

import concourse.bass as bass
from concourse import bacc
import concourse.mybir as mybir
import concourse.tile as tile
from concourse.bass import ds, ts
from concourse.bass_utils import run_bass_kernel_spmd

F32 = mybir.dt.float32
F32R = mybir.dt.float32r
F16 = mybir.dt.float16
AFT = mybir.ActivationFunctionType

B = 2
S = 2048
D = 768
H = 12
HD = 64
NH = 3  # heads per core
KT_D = D // 128  # 6 contraction tiles over d_model

LAST_EXEC_TIME_NS = None
LAST_RESULTS = None


def build_nc(nk_t):
    n_k = nk_t * 128
    nkc = (n_k + 511) // 512  # xkT 512-col chunks
    nkp = nkc * 512  # padded key columns
    nc = bacc.Bacc(None, target_bir_lowering=False)

    d_xT = nc.dram_tensor("xT", (128, 2, KT_D, 1024), F16, kind="ExternalInput")
    d_xkT = nc.dram_tensor("xkT", (128, nkc, KT_D, 512), F16, kind="ExternalInput")
    d_wq = nc.dram_tensor("wq", (128, KT_D, 192), F16, kind="ExternalInput")
    d_wk = nc.dram_tensor("wk", (128, KT_D, 192), F16, kind="ExternalInput")
    d_wv = nc.dram_tensor("wv", (128, KT_D, 192), F16, kind="ExternalInput")
    d_wo = nc.dram_tensor("wo", (128, 2, D), F16, kind="ExternalInput")
    d_bq = nc.dram_tensor("bq", (128, 2), F32, kind="ExternalInput")
    d_bk = nc.dram_tensor("bk", (128, 2), F32, kind="ExternalInput")
    d_vcol = nc.dram_tensor("vcol", (128, nk_t), F32R, kind="ExternalInput")
    d_out = nc.dram_tensor("out", (128, 16, D), F16, kind="ExternalOutput")

    with ExitStack() as ctx:
        tc = ctx.enter_context(tile.TileContext(nc))
        singles = ctx.enter_context(tc.tile_pool(name="singles", bufs=1))
        expp = ctx.enter_context(tc.tile_pool(name="expp", bufs=3))
        outp = ctx.enter_context(tc.tile_pool(name="outp", bufs=2))
        rowp = ctx.enter_context(tc.tile_pool(name="rowp", bufs=2))
        psp = ctx.enter_context(tc.tile_pool(name="psp", bufs=2, space="PSUM"))
        oaccp = ctx.enter_context(tc.tile_pool(name="oaccp", bufs=2, space="PSUM"))

        # Pull the Exp activation table load off the critical path.
        dummy = singles.tile([1, 2], F32)
        nc.vector.memset(dummy, 0.0)
        nc.scalar.activation(dummy, dummy, AFT.Exp)

        sb_xT = singles.tile([128, 2, KT_D, 1024], F16)
        sb_xkT = singles.tile([128, nkc, KT_D, 512], F16)
        sb_wq = singles.tile([128, KT_D, 192], F16)
        sb_wk = singles.tile([128, KT_D, 192], F16)
        sb_wv = singles.tile([128, KT_D, 192], F16)
        sb_wo = singles.tile([128, 2, D], F16)
        sb_bq = singles.tile([128, 2], F32)
        sb_bk = singles.tile([128, 2], F32)
        sb_v = singles.tile([128, NH, nk_t, 65], F32R)
        sb_qT = singles.tile([128, 2, S], F16)
        sb_kT = singles.tile([128, 2, nkp], F16)
        sb_onT = singles.tile([128, 2, S], F16)

        # ---- DMA: scalar queue feeds the k/v path (its exp work starts
        # later); gpsimd the small weights; sync the big xT + output.
        nc.scalar.dma_start(out=sb_wk, in_=d_wk[:, :, :])
        for c in range(nkc):
            nc.scalar.dma_start(out=sb_xkT[:, c], in_=d_xkT[:, c])
        nc.gpsimd.dma_start(out=sb_wv, in_=d_wv[:, :, :])
        nc.gpsimd.dma_start(out=sb_wq, in_=d_wq[:, :, :])
        nc.sync.dma_start(out=sb_bq, in_=d_bq[:, :])
        nc.sync.dma_start(out=sb_bk, in_=d_bk[:, :])
        for h in range(NH):
            nc.sync.dma_start(out=sb_v[:, h, :, 64:65], in_=d_vcol[:, :])
        nc.sync.dma_start(out=sb_xT[:, 0], in_=d_xT[:, 0])
        nc.sync.dma_start(out=sb_xT[:, 1], in_=d_xT[:, 1])
        nc.sync.dma_start(out=sb_wo, in_=d_wo[:, :, :])

        # ---- k projection (512-col chunks) ----
        for c in range(nkc):
            for m, rows in enumerate((128, 64)):
                msl = ds(m * 128, rows)
                ps = psp.tile([128, 512], F32, tag="ps")
                for kt in range(KT_D):
                    nc.tensor.matmul(
                        ps[:rows, :],
                        sb_wk[:, kt, msl],
                        sb_xkT[:, c, kt, :],
                        start=(kt == 0),
                        stop=(kt == KT_D - 1),
                    )
                nc.vector.tensor_scalar_add(
                    out=sb_kT[:rows, m, ds(c * 512, 512)],
                    in0=ps[:rows, :],
                    scalar1=sb_bk[:rows, m : m + 1],
                )

        # ---- v projection (per 128-key tile, all 3 heads at once) ----
        for kt2 in range(nk_t):
            c, off = (kt2 * 128) // 512, (kt2 * 128) % 512
            ps = psp.tile([128, 192], F32, tag="ps")
            for kt in range(KT_D):
                nc.tensor.matmul(
                    ps,
                    sb_xkT[:, c, kt, ds(off, 128)],
                    sb_wv[:, kt, :],
                    start=(kt == 0),
                    stop=(kt == KT_D - 1),
                )
            nc.vector.tensor_copy(
                out=sb_v[:, :, kt2, 0:64],
                in_=ps[:, 0:192].rearrange("p (h d) -> p h d", h=NH),
            )

        # ---- q projection for one 1024-col chunk ----
        def q_proj(c):
            for m, rows in enumerate((128, 64)):
                msl = ds(m * 128, rows)
                ps = psp.tile([128, 1024], F32, tag="ps")
                for qc in range(2):
                    for kt in range(KT_D):
                        nc.tensor.matmul(
                            ps[:rows, ts(qc, 512)],
                            sb_wq[:, kt, msl],
                            sb_xT[:, c, kt, ts(qc, 512)],
                            start=(kt == 0),
                            stop=(kt == KT_D - 1),
                        )
                nc.vector.tensor_scalar_add(
                    out=sb_qT[:rows, m, ds(c * 1024, 1024)],
                    in0=ps[:rows, :],
                    scalar1=sb_bq[:rows, m : m + 1],
                )

        # ---- one attention head over one 1024-query half ----
        def attn_head(half, h):
            qrow = (h % 2) * 64
            qslot = h // 2
            oacc = oaccp.tile([65, 1024], F32, tag="oacc")
            for kt2 in range(nk_t):
                sT = psp.tile([128, 1024], F32, tag="ps")
                for qc in range(2):
                    nc.tensor.matmul(
                        sT[:, ts(qc, 512)],
                        sb_kT[ds(qrow, 64), qslot, ts(kt2, 128)],
                        sb_qT[ds(qrow, 64), qslot, ds(half * 1024 + qc * 512, 512)],
                        start=True,
                        stop=True,
                    )
                et = expp.tile([128, 1024], F32R, tag="exp")
                nc.scalar.activation(et, sT, AFT.Exp)
                for qc in range(2):
                    nc.tensor.matmul(
                        oacc[:, ts(qc, 512)],
                        sb_v[:, h, kt2, :],
                        et[:, ts(qc, 512)],
                        start=(kt2 == 0),
                        stop=(kt2 == nk_t - 1),
                    )
            # normalize: o.T[0:64] / denom (row 64), into sb_onT
            drow = rowp.tile([1, 1024], F32, tag="drow")
            nc.vector.tensor_copy(out=drow, in_=oacc[64:65, :])
            rrow = rowp.tile([1, 1024], F32, tag="rrow")
            nc.vector.reciprocal_approx_fast(out=rrow, in_=drow)
            bcast = rowp.tile([64, 1024], F32, tag="bcast")
            nc.gpsimd.partition_broadcast(bcast, rrow)
            nc.vector.tensor_mul(
                out=sb_onT[ds(qrow, 64), qslot, ds(half * 1024, 1024)],
                in0=oacc[0:64, :],
                in1=bcast,
            )

        # ---- output projection for one half (two 4-tile DMA groups) ----
        def oproj(half):
            for g in range(2):
                og = outp.tile([128, 4, D], F16, tag="og")
                for j in range(4):
                    qt = half * 8 + g * 4 + j
                    ps = psp.tile([128, 1024], F32, tag="ps")
                    for eoff, ech in ((0, 512), (512, 256)):
                        nc.tensor.matmul(
                            ps[:, ds(eoff, ech)],
                            sb_onT[:, 0, ts(qt, 128)],
                            sb_wo[:, 0, ds(eoff, ech)],
                            start=True,
                            stop=False,
                        )
                        nc.tensor.matmul(
                            ps[:, ds(eoff, ech)],
                            sb_onT[0:64, 1, ts(qt, 128)],
                            sb_wo[0:64, 1, ds(eoff, ech)],
                            start=False,
                            stop=True,
                        )
                    nc.vector.tensor_copy(out=og[:, j, :], in_=ps[:, 0:D])
                nc.sync.dma_start(
                    out=d_out[:, ds(half * 8 + g * 4, 4), :], in_=og
                )

        q_proj(0)
        attn_head(0, 0)
        q_proj(1)  # PE filler while ScalarE chews head 0's exps
        attn_head(0, 1)
        attn_head(0, 2)
        attn_head(1, 0)
        oproj(0)
        attn_head(1, 1)
        attn_head(1, 2)
        oproj(1)

    nc.compile()
    return nc


def kernel(
    hidden_states,
    complexity_scores,
    attention_mask,
    Wq,
    bq,
    Wk,
    bk,
    Wv,
    bv,
    Wo,
    bo,
    emb_table,
    comp_scaling,
):
    global LAST_EXEC_TIME_NS, LAST_RESULTS
    hs = np.asarray(hidden_states, np.float32)
    cs = np.asarray(complexity_scores).astype(np.int64)
    am = np.asarray(attention_mask)
    Wq = np.asarray(Wq, np.float32)
    bq = np.asarray(bq, np.float32)
    Wk = np.asarray(Wk, np.float32)
    bk = np.asarray(bk, np.float32)
    Wv = np.asarray(Wv, np.float32)
    bv = np.asarray(bv, np.float32)
    Wo = np.asarray(Wo, np.float32)
    bo = np.asarray(bo, np.float32)
    emb_table = np.asarray(emb_table, np.float32)
    comp_scaling = np.asarray(comp_scaling, np.float32)

    # per-head score scale (identical across batch: mean over batch of embs)
    embs = emb_table[cs]  # (B, H)
    scal = comp_scaling * embs.mean(axis=0)  # (H,)
    c = (scal / math.sqrt(HD)).astype(np.float32)

    # gather unmasked keys per batch; pad to a common multiple of 128
    idx = [np.nonzero(am[b] != 0)[0] for b in range(B)]
    n_max = max(1, max(len(i) for i in idx))
    nk_t = max(2, (n_max + 127) // 128)
    n_k = nk_t * 128
    nkc = (n_k + 511) // 512
    nkp = nkc * 512

    def pack_xT(x):  # (S_or_nk, D) -> (128, chunks, KT_D, width) fp16
        xt = np.zeros((D, x.shape[1]), np.float16)
        xt[:, :] = x
        return xt

    xT = []
    xkT = []
    vcol = []
    for b in range(B):
        t = hs[b].T.astype(np.float16)  # (768, 2048)
        xT.append(
            np.ascontiguousarray(
                t.reshape(KT_D, 128, 2, 1024).transpose(1, 2, 0, 3)
            )
        )
        tk = np.zeros((D, nkp), np.float16)
        tk[:, : len(idx[b])] = hs[b][idx[b]].T
        xkT.append(
            np.ascontiguousarray(
                tk.reshape(KT_D, 128, nkc, 512).transpose(1, 2, 0, 3)
            )
        )
        v = np.zeros((nk_t * 128,), np.float32)
        v[: len(idx[b])] = 1.0
        vcol.append(np.ascontiguousarray(v.reshape(nk_t, 128).T))

    WqT = Wq.T  # (d_in, e_out)
    WkT = Wk.T
    WvT = Wv.T
    WoT = np.ascontiguousarray(Wo.T)  # rows = attended feature d

    def pack_w(w192):  # (768, 192) -> (128, KT_D, 192)
        return np.ascontiguousarray(
            w192.astype(np.float16).reshape(KT_D, 128, 192).transpose(1, 0, 2)
        )

    def pack_bias(vec):  # (192,) -> (128, 2)
        out = np.zeros((128, 2), np.float32)
        out[:, 0] = vec[:128]
        out[:64, 1] = vec[128:]
        return out

    in_maps = []
    for core in range(8):
        b = core // 4
        heads = [3 * (core % 4) + j for j in range(NH)]
        cols = np.concatenate([np.arange(h * HD, (h + 1) * HD) for h in heads])
        cscale = np.repeat(c[heads], HD)  # (192,)
        wq_c = pack_w(WqT[:, cols] * cscale[None, :])
        bq_c = bq[cols] * cscale
        wk_c = pack_w(WkT[:, cols])
        bk_c = bk[cols]
        wv_c = pack_w(WvT[:, cols])
        wo_c = np.zeros((128, 2, D), np.float16)
        wo_c[:, 0, :] = WoT[cols[:128], :]
        wo_c[:64, 1, :] = WoT[cols[128:], :]
        in_maps.append(
            {
                "xT": xT[b],
                "xkT": xkT[b],
                "wq": wq_c,
                "wk": wk_c,
                "wv": wv_c,
                "wo": np.ascontiguousarray(wo_c),
                "bq": pack_bias(bq_c),
                "bk": pack_bias(bk_c),
                "vcol": vcol[b],
            }
        )

    nc = build_nc(nk_t)
    trace = os.environ.get("KERNEL_TRACE", "0") == "1"
    res = run_bass_kernel_spmd(nc, in_maps, core_ids=list(range(8)), trace=trace)
    LAST_EXEC_TIME_NS = res.exec_time_ns
    LAST_RESULTS = res

    bo_eff = (bo + Wo @ bv).astype(np.float64)
    out = np.empty((B, S, D), np.float32)
    for b in range(B):
        acc = np.zeros((S, D), np.float64)
        for g in range(4):
            p = res.results[4 * b + g]["out"]  # (128, 16, D) fp16
            acc += p.astype(np.float64).transpose(1, 0, 2).reshape(S, D)
        out[b] = (acc + bo_eff[None, :]).astype(np.float32)
    return out
